# revision 6
# baseline (speedup 1.0000x reference)
"""Trainium2 Bass kernel for nn_Net_SLSTM: conv1d -> spiking LSTM -> BN ->
spiking LSTM -> mean -> fc, data-parallel over the batch dim (L=1024) on 8
NeuronCores.

Self-contained: takes FULL inputs, shards internally, returns FULL output.
"""
import numpy as np
from contextlib import ExitStack

import concourse.bass as bass
import concourse.mybir as mybir
import concourse.tile as tile
from concourse import bacc
from concourse.bass_utils import run_bass_kernel_spmd

F32 = mybir.dt.float32
AO = mybir.AluOpType
AF = mybir.ActivationFunctionType

# Problem shapes (hardcoded per the contract)
T, L, C, H, NCLS = 256, 1024, 14, 128, 7
N_CORES = 8
B = L // N_CORES          # 128 batch rows per core
CP = 16                   # conv input channels padded 14 -> 16
G4 = 4 * H                # 512

# Tunables
NCHAINS = 2               # interleaved sub-batch chains per core
BP = B // NCHAINS
KSTAGE = 4                # spike steps staged per DMA to DRAM
XCHUNK = 16               # timesteps of x per input DMA
RING0 = 16                # spk0 ring slots (timesteps)
SRING = 8                 # spike staging ring slots (multiple of KSTAGE)
PFB = 6                   # spk1 prefetch buffers (phase 2)
BN_EPS = 1e-5

_prog_cache = {}


def _emit_lstm_step(nc, ch, t, st, cfg):
    """Emit one LSTM step for chain `ch` at time t.

    st: per-chain mutable state dict with keys
      mem (AP), spk (AP), vgsyn, sring, bn, acc2, pools...
    cfg: dict with phase config: wx(lhsT [*,512]), wh, rhs_fn(t, ch)->AP,
      thr, is_l2, mask4/bias2k4 for l2, spk1_dram for l1 store.
    """
    ps = st["pspool"].tile([128, 4 * BP], F32, tag=f"ps{ch}")
    if cfg["is_l2"]:
        # bias via K=4 matmul covering the whole [128, 4*BP] bank
        nc.tensor.matmul(ps[:], cfg["bias2k4"][:], cfg["mask4"][:],
                         start=True, stop=False)
    for c in range(4):
        sl = ps[:, c * BP:(c + 1) * BP]
        nc.tensor.matmul(sl, cfg["wx"][:, c * H:(c + 1) * H], cfg["rhs"],
                         start=not cfg["is_l2"], stop=False)
        nc.tensor.matmul(sl, cfg["wh"][:, c * H:(c + 1) * H], st["mem"],
                         start=False, stop=(c == 3))
    # u = sigmoid over all 4 gate blocks (g pre-scaled by 2 on host)
    u = st["upool"].tile([128, 4 * BP], F32, tag=f"u{ch}")
    nc.scalar.activation(u[:], ps[:], AF.Sigmoid)
    vgsyn = st["vgsyn"]
    # vg = 2*u_g - 1  (= tanh(g))
    nc.vector.tensor_scalar(vgsyn[:, 0:BP], u[:, 2 * BP:3 * BP],
                            2.0, -1.0, op0=AO.mult, op1=AO.add)
    # [t1|t2] = [u_i|u_f] * [vg|syn]
    t12 = st["t12pool"].tile([128, 2 * BP], F32, tag=f"t12{ch}")
    nc.vector.tensor_tensor(t12[:], u[:, 0:2 * BP], vgsyn[:, 0:2 * BP],
                            op=AO.mult)
    # syn' = t1 + t2 (written into the persistent syn slot)
    nc.vector.tensor_tensor(vgsyn[:, BP:2 * BP], t12[:, 0:BP],
                            t12[:, BP:2 * BP], op=AO.add)
    w = st["wpool"].tile([128, BP], F32, tag=f"w{ch}")
    nc.scalar.activation(w[:], vgsyn[:, BP:2 * BP], AF.Tanh)
    m1 = st["m1pool"].tile([128, BP], F32, tag=f"m1{ch}")
    nc.vector.tensor_tensor(m1[:], u[:, 3 * BP:4 * BP], w[:], op=AO.mult)
    # mem' = m1 - spk_prev ({0,thr})
    mem_new = st["mempool"].tile([128, BP], F32, tag=f"mem{ch}")
    nc.vector.tensor_tensor(mem_new[:], m1[:], st["spk"], op=AO.subtract)
    # spk = (m1 - thr) > spk_prev   [== mem' > thr for thr=1]
    slot = t % SRING
    spk_new = st["sring"][:, slot * BP:(slot + 1) * BP]
    kw = {}
    if st.get("bn") is not None:
        kw["accum_out"] = st["bn"][:, t:t + 1]
    nc.vector.scalar_tensor_tensor(spk_new, m1[:], cfg["thr"], st["spk"],
                                   op0=AO.subtract, op1=AO.is_gt, **kw)
    st["mem"] = mem_new[:]
    st["spk"] = spk_new
    if cfg.get("spk1_dram") is not None and (t + 1) % KSTAGE == 0:
        s0 = (t + 1 - KSTAGE) % SRING
        src = st["sring"][:, s0 * BP:(s0 + KSTAGE) * BP]
        dst = cfg["spk1_dram"][:, t + 1 - KSTAGE:t + 1,
                               ch * BP:(ch + 1) * BP]
        nc.sync.dma_start(dst, src.rearrange("p (s b) -> p s b", b=BP))
    if cfg["is_l2"]:
        nc.vector.tensor_tensor(st["acc2"][:], st["acc2"][:], mem_new[:],
                                op=AO.add)
    if cfg.get("dbg") is not None and t == 0 and ch == 0:
        d = cfg["dbg"]
        nc.sync.dma_start(d["u0"][:], u[:])
        nc.sync.dma_start(d["vgsyn0"][:], vgsyn[:])
        nc.sync.dma_start(d["mem0"][:], mem_new[:])


def build_program(thr1, thr2, t_run):
    nc = bacc.Bacc("TRN2", target_bir_lowering=False, debug=False,
                   num_devices=N_CORES)
    # ---- dram I/O ----
    xT_d = nc.dram_tensor("xT", [T, CP, B + 2], F32, kind="ExternalInput")
    convw_d = nc.dram_tensor("convw", [CP, 96], F32, kind="ExternalInput")
    thr0_d = nc.dram_tensor("thr0", [32, 1], F32, kind="ExternalInput")
    wx1_d = nc.dram_tensor("wx1", [33, G4], F32, kind="ExternalInput")
    wh1_d = nc.dram_tensor("wh1", [H, G4], F32, kind="ExternalInput")
    wx2_d = nc.dram_tensor("wx2", [H, G4], F32, kind="ExternalInput")
    wh2_d = nc.dram_tensor("wh2", [H, G4], F32, kind="ExternalInput")
    bsum2_d = nc.dram_tensor("bsum2", [1, G4], F32, kind="ExternalInput")
    mask4_d = nc.dram_tensor("mask4", [4, 4 * BP], F32, kind="ExternalInput")
    gamma_d = nc.dram_tensor("gamma", [H, 1], F32, kind="ExternalInput")
    beta_d = nc.dram_tensor("beta", [H, 1], F32, kind="ExternalInput")
    acc2_d = nc.dram_tensor("acc2", [H, B], F32, kind="ExternalOutput")
    bnsum_d = nc.dram_tensor("bnsum", [H, 1], F32, kind="ExternalOutput")
    ccw_d = nc.dram_tensor("ccw", [H, 1], F32, kind="ExternalOutput")
    dbg_u0 = nc.dram_tensor("dbg_u0", [128, 4 * BP], F32, kind="ExternalOutput")
    dbg_vgsyn0 = nc.dram_tensor("dbg_vgsyn0", [128, 2 * BP], F32, kind="ExternalOutput")
    dbg_mem0 = nc.dram_tensor("dbg_mem0", [128, BP], F32, kind="ExternalOutput")
    dbg_b4 = nc.dram_tensor("dbg_b4", [4, H], F32, kind="ExternalOutput")
    dbg_pf0 = nc.dram_tensor("dbg_pf0", [128, B], F32, kind="ExternalOutput")

    with ExitStack() as ctx:
        tc = ctx.enter_context(tile.TileContext(nc))
        P = lambda name, bufs, **kw: ctx.enter_context(
            tc.tile_pool(name=name, bufs=bufs, **kw))
        persist = P("persist", 1)
        dram = P("dram", 1, space="DRAM")
        xpool = P("xpool", 3)
        pfpool = P("pfpool", PFB)
        pspool = P("pspool", 2, space="PSUM")
        psc = P("psc", 2, space="PSUM")
        psb = P("psb", 1, space="PSUM")
        upool = P("upool", 2)
        t12pool = P("t12pool", 2)
        wpool = P("wpool", 2)
        m1pool = P("m1pool", 2)
        mempool = P("mempool", 2)
        tiny = P("tiny", 1)

        # ---- persistent SBUF ----
        convw = persist.tile([CP, 96], F32, tag="convw")
        thr0 = persist.tile([32, 1], F32, tag="thr0")
        wx1 = persist.tile([33, G4], F32, tag="wx1")
        wh1 = persist.tile([H, G4], F32, tag="wh1")
        wx2r = persist.tile([H, G4], F32, tag="wx2r")
        wx2s = persist.tile([H, G4], F32, tag="wx2s")
        wh2 = persist.tile([H, G4], F32, tag="wh2")
        bsum2 = persist.tile([1, G4], F32, tag="bsum2")
        mask4 = persist.tile([4, 4 * BP], F32, tag="mask4")
        gamma = persist.tile([H, 1], F32, tag="gamma")
        beta = persist.tile([H, 1], F32, tag="beta")
        bias2k4 = persist.tile([4, H], F32, tag="bias2k4")
        s0ring = persist.tile([33, RING0 * B], F32, tag="s0ring")
        zt = persist.tile([128, BP], F32, tag="zt")
        spk1_dram = dram.tile([H, T, B], F32)

        for dst, src in [(convw, convw_d), (thr0, thr0_d), (wx1, wx1_d),
                         (wh1, wh1_d), (wx2r, wx2_d), (wh2, wh2_d),
                         (bsum2, bsum2_d), (mask4, mask4_d),
                         (gamma, gamma_d), (beta, beta_d)]:
            nc.sync.dma_start(dst[:], src[:])
        nc.gpsimd.memset(zt[:], 0.0)
        nc.gpsimd.memset(s0ring[32:33, :], 1.0)

        # warm up the collectives path early (result -> ccw output)
        ccin = dram.tile([H, 1], F32)
        ccout = dram.tile([H, 1], F32)
        ccs = tiny.tile([H, 1], F32, tag="ccs")
        nc.gpsimd.memset(ccs[:], 0.0)
        nc.sync.dma_start(ccin[:], ccs[:])
        nc.gpsimd.collective_compute(
            "AllReduce", AO.add, replica_groups=[list(range(N_CORES))],
            ins=[ccin[:]], outs=[ccout[:]])
        nc.sync.dma_start(ccw_d[:], ccout[:])

        # ---- per-chain state ----
        chains = []
        for ch in range(NCHAINS):
            st = dict(pspool=pspool, upool=upool, t12pool=t12pool,
                      wpool=wpool, m1pool=m1pool, mempool=mempool)
            st["vgsyn"] = persist.tile([128, 2 * BP], F32, tag=f"vgsyn{ch}", name=f"vgsyn{ch}")
            st["sring"] = persist.tile([128, SRING * BP], F32, tag=f"sr{ch}", name=f"sr{ch}")
            st["bn"] = persist.tile([128, T], F32, tag=f"bn{ch}", name=f"bn{ch}")
            st["acc2"] = persist.tile([128, BP], F32, tag=f"acc2{ch}", name=f"acc2{ch}")
            nc.gpsimd.memset(st["vgsyn"][:, BP:2 * BP], 0.0)
            nc.gpsimd.memset(st["acc2"][:], 0.0)
            st["mem"] = zt[:]
            st["spk"] = zt[:]
            chains.append(st)

        # ---- phase 1: conv + LSTM1 ----
        cfg1 = dict(wx=wx1, wh=wh1, thr=float(thr1), is_l2=False,
                    spk1_dram=spk1_dram)
        xt = None
        for t in range(t_run):
            if t % XCHUNK == 0:
                xt = xpool.tile([CP, XCHUNK, B + 2], F32, tag="xt")
                nc.sync.dma_start(
                    xt[:], xT_d[t:t + XCHUNK].rearrange("t c l -> c t l"))
            tt = t % XCHUNK
            pc = psc.tile([32, B], F32, tag="pc")
            for k in range(3):
                nc.tensor.matmul(pc[:], convw[:, k * 32:(k + 1) * 32],
                                 xt[:, tt, k:k + B],
                                 start=(k == 0), stop=(k == 2))
            slot0 = t % RING0
            nc.vector.tensor_scalar(
                s0ring[0:32, slot0 * B:(slot0 + 1) * B], pc[:], thr0[:],
                None, op0=AO.is_gt)
            for ch, st in enumerate(chains):
                cfg1["rhs"] = s0ring[0:33, slot0 * B + ch * BP:
                                     slot0 * B + (ch + 1) * BP]
                _emit_lstm_step(nc, ch, t, st, cfg1)

        # ---- BN stats + allreduce + weight fold ----
        r = tiny.tile([H, 1], F32, tag="r0")
        nc.vector.tensor_reduce(r[:], chains[0]["bn"][:, 0:t_run],
                                mybir.AxisListType.X, AO.add)
        for st in chains[1:]:
            r2 = tiny.tile([H, 1], F32, tag="r1")
            nc.vector.tensor_reduce(r2[:], st["bn"][:, 0:t_run],
                                    mybir.AxisListType.X, AO.add)
            nc.vector.tensor_tensor(r[:], r[:], r2[:], op=AO.add)
        bnin = dram.tile([H, 1], F32)
        bnout = dram.tile([H, 1], F32)
        nc.sync.dma_start(bnin[:], r[:])
        nc.gpsimd.collective_compute(
            "AllReduce", AO.add, replica_groups=[list(range(N_CORES))],
            ins=[bnin[:]], outs=[bnout[:]])
        stot = tiny.tile([H, 1], F32, tag="stot")
        nc.sync.dma_start(stot[:], bnout[:])
        nc.sync.dma_start(bnsum_d[:], bnout[:])
        # mu, var = mu*(1-mu); a = gamma/sqrt(var+eps); c = beta - mu*a
        mu = tiny.tile([H, 1], F32, tag="mu")
        nc.vector.tensor_scalar_mul(mu[:], stot[:], 1.0 / (t_run * L))
        om = tiny.tile([H, 1], F32, tag="om")
        nc.vector.tensor_scalar(om[:], mu[:], -1.0, 1.0,
                                op0=AO.mult, op1=AO.add)
        var = tiny.tile([H, 1], F32, tag="var")
        nc.vector.tensor_tensor(var[:], mu[:], om[:], op=AO.mult)
        xve = tiny.tile([H, 1], F32, tag="xve")
        nc.vector.tensor_scalar_add(xve[:], var[:], BN_EPS)
        epsb = tiny.tile([H, 1], F32, tag="epsb")
        nc.gpsimd.memset(epsb[:], BN_EPS)
        y1 = tiny.tile([H, 1], F32, tag="y1")
        nc.scalar.activation(y1[:], var[:], AF.Sqrt, bias=epsb[:])
        # one Newton step: y2 = 0.5*(y1 + x/y1)
        ry = tiny.tile([H, 1], F32, tag="ry")
        nc.vector.reciprocal(ry[:], y1[:])
        z = tiny.tile([H, 1], F32, tag="z")
        nc.vector.tensor_tensor(z[:], xve[:], ry[:], op=AO.mult)
        y2 = tiny.tile([H, 1], F32, tag="y2")
        nc.vector.tensor_tensor(y2[:], y1[:], z[:], op=AO.add)
        nc.vector.tensor_scalar_mul(y2[:], y2[:], 0.5)
        rinv = tiny.tile([H, 1], F32, tag="rinv")
        nc.vector.reciprocal(rinv[:], y2[:])
        a = tiny.tile([H, 1], F32, tag="a")
        nc.vector.tensor_tensor(a[:], gamma[:], rinv[:], op=AO.mult)
        cm = tiny.tile([H, 1], F32, tag="cm")
        nc.vector.tensor_tensor(cm[:], mu[:], a[:], op=AO.mult)
        cvec = tiny.tile([H, 1], F32, tag="cvec")
        nc.vector.tensor_tensor(cvec[:], beta[:], cm[:], op=AO.subtract)
        # wx2s = wx2r * a (per-partition); bias row = c^T wx2r + bsum2
        nc.vector.tensor_scalar_mul(wx2s[:], wx2r[:], a[:])
        pb = psb.tile([1, G4], F32, tag="pb")
        nc.tensor.matmul(pb[:], cvec[:], wx2r[:], start=True, stop=True)
        brow = tiny.tile([1, G4], F32, tag="brow")
        nc.vector.tensor_tensor(brow[:], pb[:], bsum2[:], op=AO.add)
        for k4 in range(4):
            nc.sync.dma_start(bias2k4[k4:k4 + 1, :],
                              brow[0:1, k4 * H:(k4 + 1) * H])

        # ---- phase 2: LSTM2 ----
        for ch, st in enumerate(chains):
            nc.gpsimd.memset(st["vgsyn"][:, BP:2 * BP], 0.0)
            st["mem"] = zt[:]
            st["spk"] = zt[:]
            st["bn"] = None
        cfg2 = dict(wx=wx2s, wh=wh2, thr=float(thr2), is_l2=True,
                    mask4=mask4, bias2k4=bias2k4, spk1_dram=None,
                    dbg=dict(u0=dbg_u0, vgsyn0=dbg_vgsyn0, mem0=dbg_mem0))
        nc.sync.dma_start(dbg_b4[:], bias2k4[:])
        for t in range(t_run):
            pf = pfpool.tile([128, B], F32, tag="pf")
            nc.sync.dma_start(pf[:], spk1_dram[:, t, :])
            if t == 0:
                nc.sync.dma_start(dbg_pf0[:], pf[:])
            for ch, st in enumerate(chains):
                cfg2["rhs"] = pf[:, ch * BP:(ch + 1) * BP]
                _emit_lstm_step(nc, ch, t, st, cfg2)
        for ch, st in enumerate(chains):
            nc.sync.dma_start(acc2_d[:, ch * BP:(ch + 1) * BP],
                              st["acc2"][:])
    nc.compile()
    return nc


def _prep_host(inputs, t_run):
    """Build per-core input maps from full inputs."""
    x = np.asarray(inputs["x"], np.float32)
    conv_w = np.asarray(inputs["conv_w"], np.float32)
    conv_b = np.asarray(inputs["conv_b"], np.float32)

    def gscale(row512):
        r = row512.copy()
        r[..., 2 * H:3 * H] *= 2.0
        return r

    wx1 = np.concatenate(
        [np.asarray(inputs["w_ih1"], np.float32).T,
         (np.asarray(inputs["b_ih1"], np.float32)
          + np.asarray(inputs["b_hh1"], np.float32))[None, :]], axis=0)
    wx1 = gscale(wx1)
    wh1 = gscale(np.asarray(inputs["w_hh1"], np.float32).T)
    wx2 = gscale(np.asarray(inputs["w_ih2"], np.float32).T)
    wh2 = gscale(np.asarray(inputs["w_hh2"], np.float32).T)
    bsum2 = gscale((np.asarray(inputs["b_ih2"], np.float32)
                    + np.asarray(inputs["b_hh2"], np.float32))[None, :])
    convw = np.zeros((CP, 96), np.float32)
    for k in range(3):
        convw[:C, k * 32:(k + 1) * 32] = conv_w[:, :, k].T
    thr0 = (1.0 - conv_b)[:, None].astype(np.float32)
    mask4 = np.kron(np.eye(4, dtype=np.float32),
                    np.ones((1, BP), np.float32))
    gamma = np.asarray(inputs["bn_gamma"], np.float32)[:, None]
    beta = np.asarray(inputs["bn_beta"], np.float32)[:, None]

    xp = np.zeros((T, L + 2, C), np.float32)
    xp[:, 1:L + 1, :] = x
    in_maps = []
    for k in range(N_CORES):
        xk = xp[:, k * B:k * B + B + 2, :]          # [T, B+2, C]
        xTk = np.zeros((T, CP, B + 2), np.float32)
        xTk[:, :C, :] = np.ascontiguousarray(xk.transpose(0, 2, 1))
        in_maps.append(dict(
            xT=xTk, convw=convw, thr0=thr0, wx1=np.ascontiguousarray(wx1),
            wh1=np.ascontiguousarray(wh1), wx2=np.ascontiguousarray(wx2),
            wh2=np.ascontiguousarray(wh2),
            bsum2=np.ascontiguousarray(bsum2), mask4=mask4,
            gamma=gamma, beta=beta))
    return in_maps


def run(inputs, t_run=T, trace=False):
    thr1 = float(np.asarray(inputs["thr1"]))
    thr2 = float(np.asarray(inputs["thr2"]))
    key = (thr1, thr2, t_run, NCHAINS)
    if key not in _prog_cache:
        _prog_cache[key] = build_program(thr1, thr2, t_run)
    nc = _prog_cache[key]
    in_maps = _prep_host(inputs, t_run)
    res = run_bass_kernel_spmd(nc, in_maps, core_ids=list(range(N_CORES)),
                               trace=trace)
    acc2 = np.concatenate([res.results[k]["acc2"] for k in range(N_CORES)],
                          axis=1)                    # [H, L]
    final_mem = acc2.T / float(t_run)                # [L, H]
    fc_w = np.asarray(inputs["fc_w"], np.float32)
    fc_b = np.asarray(inputs["fc_b"], np.float32)
    out = final_mem @ fc_w.T + fc_b
    return out.astype(np.float32), res


def kernel(**inputs):
    out, _ = run(inputs)
    return out


# revision 7
# speedup vs baseline: 2.4684x; 2.4684x over previous
"""Trainium2 Bass kernel for nn_Net_SLSTM: conv1d -> spiking LSTM -> BN ->
spiking LSTM -> mean -> fc, data-parallel over the batch dim (L=1024) on 8
NeuronCores.

Self-contained: takes FULL inputs, shards internally, returns FULL output.

Key facts used:
- SLSTM with thr=1: mem = sig(o)*tanh(syn) in (-1,1) strictly, so the
  LSTM layers can never spike and reset is always 0. The reset subtract
  is dropped (exact); layer-1 spikes are still computed for the BN
  statistics (they come out zero, but honestly so).
- Because layer-1 spikes are structurally zero, everything in phase 1
  only influences the output through those (zero) spike counts, so all
  of phase 1 runs in bf16. Phase 2's recurrence does affect the output;
  bf16 matmuls there cost ~1.7e-3 relative error (measured), elementwise
  stays fp32.
- BN over {0,1} spikes: var = mu*(1-mu), so only per-H spike counts are
  needed -> one tiny AllReduce, folded into layer-2's weights/bias.
"""
import numpy as np
from contextlib import ExitStack

import ml_dtypes
import concourse.bass as bass
import concourse.mybir as mybir
import concourse.tile as tile
from concourse import bacc
from concourse.bass_utils import run_bass_kernel_spmd

F32 = mybir.dt.float32
BF16 = mybir.dt.bfloat16
AO = mybir.AluOpType
AF = mybir.ActivationFunctionType

# Problem shapes (hardcoded per the contract)
T, L, C, H, NCLS = 256, 1024, 14, 128, 7
N_CORES = 8
B = L // N_CORES          # 128 batch rows per core
CP = 16                   # conv input channels padded 14 -> 16
G4 = 4 * H                # 512

# Tunables
NCHAINS = 2               # interleaved sub-batch chains per core
BP = B // NCHAINS
KSTAGE = 4                # spike steps staged per DMA to DRAM
XCHUNK = 16               # timesteps of x per input DMA
RING0 = 16                # spk0 ring slots (timesteps)
SRING = 8                 # spike staging ring slots (multiple of KSTAGE)
PFB = 6                   # spk1 prefetch buffers (phase 2)
BN_EPS = 1e-5

_prog_cache = {}


def _emit_lstm_step(nc, ch, t, st, cfg):
    """One LSTM step for chain `ch` at time t (transposed [H, B] layout)."""
    edt = cfg["edt"]
    ps = st["pspool"].tile([128, 4 * BP], F32, tag=f"ps{ch}", name=f"ps{ch}")
    if cfg["is_l2"]:
        nc.tensor.matmul(ps[:], cfg["bias2k4"][:], cfg["mask4"][:],
                         start=True, stop=False)
    for c in range(4):
        sl = ps[:, c * BP:(c + 1) * BP]
        nc.tensor.matmul(sl, cfg["wx"][:, c * H:(c + 1) * H], cfg["rhs"],
                         start=not cfg["is_l2"], stop=False)
        nc.tensor.matmul(sl, cfg["wh"][:, c * H:(c + 1) * H], st["mem"],
                         start=False, stop=(c == 3))
    # u = sigmoid over all 4 gate blocks (g pre-scaled by 2 on host)
    u = st["upool"].tile([128, 4 * BP], edt, tag=f"u{ch}", name=f"u{ch}")
    nc.scalar.activation(u[:], ps[:], AF.Sigmoid)
    vgsyn = st["vgsyn"]
    # vg = 2*u_g - 1  (= tanh(g))
    nc.vector.tensor_scalar(vgsyn[:, 0:BP], u[:, 2 * BP:3 * BP],
                            2.0, -1.0, op0=AO.mult, op1=AO.add)
    # [t1|t2] = [u_i|u_f] * [vg|syn]
    t12 = st["t12pool"].tile([128, 2 * BP], edt, tag=f"t12{ch}",
                             name=f"t12{ch}")
    nc.vector.tensor_tensor(t12[:], u[:, 0:2 * BP], vgsyn[:, 0:2 * BP],
                            op=AO.mult)
    # syn' = t1 + t2 (into the persistent syn slot)
    nc.vector.tensor_tensor(vgsyn[:, BP:2 * BP], t12[:, 0:BP],
                            t12[:, BP:2 * BP], op=AO.add)
    w = st["wpool"].tile([128, BP], edt, tag=f"w{ch}", name=f"w{ch}")
    nc.scalar.activation(w[:], vgsyn[:, BP:2 * BP], AF.Tanh)
    # mem' = sig(o)*tanh(syn')   (reset is provably always zero)
    m1 = st["m1pool"].tile([128, BP], BF16, tag=f"m1{ch}", name=f"m1{ch}")
    nc.vector.tensor_tensor(m1[:], u[:, 3 * BP:4 * BP], w[:], op=AO.mult)
    st["mem"] = m1[:]
    if not cfg["is_l2"]:
        # spike = (mem > thr) -> {1,0} bf16, into the staging ring (gpsimd)
        slot = t % SRING
        spk_new = st["sring"][:, slot * BP:(slot + 1) * BP]
        nc.gpsimd.tensor_scalar(spk_new, m1[:], cfg["thr"], None,
                                op0=AO.is_gt)
        if (t + 1) % KSTAGE == 0:
            s0 = (t + 1 - KSTAGE) % SRING
            src = st["sring"][:, s0 * BP:(s0 + KSTAGE) * BP]
            dst = cfg["spk1_dram"][:, t + 1 - KSTAGE:t + 1,
                                   ch * BP:(ch + 1) * BP]
            nc.sync.dma_start(dst, src.rearrange("p (s b) -> p s b", b=BP))
            # partial BN spike counts over this 4-step window
            idx = (t + 1) // KSTAGE - 1
            nc.vector.tensor_reduce(st["bnp"][:, idx:idx + 1], src,
                                    mybir.AxisListType.X, AO.add)
    else:
        nc.gpsimd.tensor_tensor(st["acc2"][:], st["acc2"][:], m1[:],
                                op=AO.add)


def build_program(thr1, thr2, t_run):
    nc = bacc.Bacc("TRN2", target_bir_lowering=False, debug=False,
                   num_devices=N_CORES)
    # ---- dram I/O ----
    xT_d = nc.dram_tensor("xT", [T, CP, B + 2], BF16, kind="ExternalInput")
    convw_d = nc.dram_tensor("convw", [CP, 96], BF16, kind="ExternalInput")
    thr0_d = nc.dram_tensor("thr0", [32, 1], F32, kind="ExternalInput")
    wx1_d = nc.dram_tensor("wx1", [33, G4], BF16, kind="ExternalInput")
    wh1_d = nc.dram_tensor("wh1", [H, G4], BF16, kind="ExternalInput")
    wx2_d = nc.dram_tensor("wx2", [H, G4], F32, kind="ExternalInput")
    wh2_d = nc.dram_tensor("wh2", [H, G4], BF16, kind="ExternalInput")
    bsum2_d = nc.dram_tensor("bsum2", [1, G4], F32, kind="ExternalInput")
    mask4_d = nc.dram_tensor("mask4", [4, 4 * BP], BF16,
                             kind="ExternalInput")
    gamma_d = nc.dram_tensor("gamma", [H, 1], F32, kind="ExternalInput")
    beta_d = nc.dram_tensor("beta", [H, 1], F32, kind="ExternalInput")
    acc2_d = nc.dram_tensor("acc2", [H, B], F32, kind="ExternalOutput")
    bnsum_d = nc.dram_tensor("bnsum", [H, 1], F32, kind="ExternalOutput")
    ccw_d = nc.dram_tensor("ccw", [H, 1], F32, kind="ExternalOutput")

    with ExitStack() as ctx:
        tc = ctx.enter_context(tile.TileContext(nc))
        P = lambda name, bufs, **kw: ctx.enter_context(
            tc.tile_pool(name=name, bufs=bufs, **kw))
        persist = P("persist", 1)
        dram = P("dram", 1, space="DRAM")
        xpool = P("xpool", 3)
        pfpool = P("pfpool", PFB)
        pspool = P("pspool", 2, space="PSUM")
        psc = P("psc", 2, space="PSUM")
        psb = P("psb", 1, space="PSUM")
        upool = P("upool", 2)
        t12pool = P("t12pool", 2)
        wpool = P("wpool", 2)
        m1pool = P("m1pool", 3)
        tiny = P("tiny", 1)

        # ---- persistent SBUF ----
        convw = persist.tile([CP, 96], BF16, tag="convw")
        thr0 = persist.tile([32, 1], F32, tag="thr0")
        wx1 = persist.tile([33, G4], BF16, tag="wx1")
        wh1 = persist.tile([H, G4], BF16, tag="wh1")
        wx2r = persist.tile([H, G4], F32, tag="wx2r")
        wx2s = persist.tile([H, G4], BF16, tag="wx2s")
        wh2 = persist.tile([H, G4], BF16, tag="wh2")
        bsum2 = persist.tile([1, G4], F32, tag="bsum2")
        mask4 = persist.tile([4, 4 * BP], BF16, tag="mask4")
        gamma = persist.tile([H, 1], F32, tag="gamma")
        beta = persist.tile([H, 1], F32, tag="beta")
        bias2k4 = persist.tile([4, H], BF16, tag="bias2k4")
        s0ring = persist.tile([33, RING0 * B], BF16, tag="s0ring")
        spk1_dram = dram.tile([H, T, B], BF16)

        for dst, src in [(convw, convw_d), (thr0, thr0_d), (wx1, wx1_d),
                         (wh1, wh1_d), (wx2r, wx2_d), (wh2, wh2_d),
                         (bsum2, bsum2_d), (mask4, mask4_d),
                         (gamma, gamma_d), (beta, beta_d)]:
            nc.sync.dma_start(dst[:], src[:])
        nc.gpsimd.memset(s0ring[32:33, :], 1.0)

        # warm up the collectives path early (result -> ccw output)
        ccin = dram.tile([H, 1], F32)
        ccout = dram.tile([H, 1], F32)
        ccs = tiny.tile([H, 1], F32, tag="ccs")
        nc.gpsimd.memset(ccs[:], 0.0)
        nc.sync.dma_start(ccin[:], ccs[:])
        nc.gpsimd.collective_compute(
            "AllReduce", AO.add, replica_groups=[list(range(N_CORES))],
            ins=[ccin[:]], outs=[ccout[:]])
        nc.sync.dma_start(ccw_d[:], ccout[:])

        # ---- per-chain state ----
        NW = t_run // KSTAGE
        chains = []
        for ch in range(NCHAINS):
            st = dict(pspool=pspool, upool=upool, t12pool=t12pool,
                      wpool=wpool, m1pool=m1pool)
            st["vgsyn1"] = persist.tile([128, 2 * BP], BF16,
                                        tag=f"vgsyn1{ch}", name=f"vgsyn1{ch}")
            st["vgsyn2"] = persist.tile([128, 2 * BP], F32,
                                        tag=f"vgsyn2{ch}", name=f"vgsyn2{ch}")
            st["sring"] = persist.tile([128, SRING * BP], BF16,
                                       tag=f"sr{ch}", name=f"sr{ch}")
            st["bnp"] = persist.tile([128, NW], F32, tag=f"bnp{ch}",
                                     name=f"bnp{ch}")
            st["acc2"] = persist.tile([128, BP], F32, tag=f"acc2{ch}",
                                      name=f"acc2{ch}")
            zt = persist.tile([128, BP], BF16, tag=f"zt{ch}",
                              name=f"zt{ch}")
            nc.gpsimd.memset(zt[:], 0.0)
            st["zt"] = zt
            nc.gpsimd.memset(st["vgsyn1"][:, BP:2 * BP], 0.0)
            nc.gpsimd.memset(st["acc2"][:], 0.0)
            st["mem"] = zt[:]
            st["vgsyn"] = st["vgsyn1"]
            chains.append(st)

        # ---- phase 1: conv + LSTM1 (all bf16) ----
        cfg1 = dict(wx=wx1, wh=wh1, thr=float(thr1), is_l2=False,
                    spk1_dram=spk1_dram, edt=BF16)
        xt = None
        for t in range(t_run):
            if t % XCHUNK == 0:
                xt = xpool.tile([CP, XCHUNK, B + 2], BF16, tag="xt",
                                name="xt")
                nc.sync.dma_start(
                    xt[:], xT_d[t:t + XCHUNK].rearrange("t c l -> c t l"))
            tt = t % XCHUNK
            pc = psc.tile([32, B], F32, tag="pc", name="pc")
            for k in range(3):
                nc.tensor.matmul(pc[:], convw[:, k * 32:(k + 1) * 32],
                                 xt[:, tt, k:k + B],
                                 start=(k == 0), stop=(k == 2))
            slot0 = t % RING0
            nc.vector.tensor_scalar(
                s0ring[0:32, slot0 * B:(slot0 + 1) * B], pc[:], thr0[:],
                None, op0=AO.is_gt)
            for ch, st in enumerate(chains):
                cfg1["rhs"] = s0ring[0:33, slot0 * B + ch * BP:
                                     slot0 * B + (ch + 1) * BP]
                _emit_lstm_step(nc, ch, t, st, cfg1)

        # ---- BN stats + allreduce + weight fold (fp32, tiny) ----
        r = tiny.tile([H, 1], F32, tag="r0")
        nc.vector.tensor_reduce(r[:], chains[0]["bnp"][:],
                                mybir.AxisListType.X, AO.add)
        for st in chains[1:]:
            r2 = tiny.tile([H, 1], F32, tag="r1")
            nc.vector.tensor_reduce(r2[:], st["bnp"][:],
                                    mybir.AxisListType.X, AO.add)
            nc.vector.tensor_tensor(r[:], r[:], r2[:], op=AO.add)
        bnin = dram.tile([H, 1], F32)
        bnout = dram.tile([H, 1], F32)
        nc.sync.dma_start(bnin[:], r[:])
        nc.gpsimd.collective_compute(
            "AllReduce", AO.add, replica_groups=[list(range(N_CORES))],
            ins=[bnin[:]], outs=[bnout[:]])
        stot = tiny.tile([H, 1], F32, tag="stot")
        nc.sync.dma_start(stot[:], bnout[:])
        nc.sync.dma_start(bnsum_d[:], bnout[:])
        mu = tiny.tile([H, 1], F32, tag="mu")
        nc.vector.tensor_scalar_mul(mu[:], stot[:], 1.0 / (t_run * L))
        om = tiny.tile([H, 1], F32, tag="om")
        nc.vector.tensor_scalar(om[:], mu[:], -1.0, 1.0,
                                op0=AO.mult, op1=AO.add)
        var = tiny.tile([H, 1], F32, tag="var")
        nc.vector.tensor_tensor(var[:], mu[:], om[:], op=AO.mult)
        xve = tiny.tile([H, 1], F32, tag="xve")
        nc.vector.tensor_scalar_add(xve[:], var[:], BN_EPS)
        epsb = tiny.tile([H, 1], F32, tag="epsb")
        nc.gpsimd.memset(epsb[:], BN_EPS)
        y1 = tiny.tile([H, 1], F32, tag="y1")
        nc.scalar.activation(y1[:], var[:], AF.Sqrt, bias=epsb[:])
        # one Newton step: y2 = 0.5*(y1 + x/y1); a = gamma/y2
        ry = tiny.tile([H, 1], F32, tag="ry")
        nc.vector.reciprocal(ry[:], y1[:])
        z = tiny.tile([H, 1], F32, tag="z")
        nc.vector.tensor_tensor(z[:], xve[:], ry[:], op=AO.mult)
        y2 = tiny.tile([H, 1], F32, tag="y2")
        nc.vector.tensor_tensor(y2[:], y1[:], z[:], op=AO.add)
        nc.vector.tensor_scalar_mul(y2[:], y2[:], 0.5)
        rinv = tiny.tile([H, 1], F32, tag="rinv")
        nc.vector.reciprocal(rinv[:], y2[:])
        a = tiny.tile([H, 1], F32, tag="a")
        nc.vector.tensor_tensor(a[:], gamma[:], rinv[:], op=AO.mult)
        cm = tiny.tile([H, 1], F32, tag="cm")
        nc.vector.tensor_tensor(cm[:], mu[:], a[:], op=AO.mult)
        cvec = tiny.tile([H, 1], F32, tag="cvec")
        nc.vector.tensor_tensor(cvec[:], beta[:], cm[:], op=AO.subtract)
        # wx2s = wx2r * a (per-partition, bf16 out)
        nc.vector.tensor_scalar_mul(wx2s[:], wx2r[:], a[:])
        pb = psb.tile([1, G4], F32, tag="pb")
        nc.tensor.matmul(pb[:], cvec[:], wx2r[:], start=True, stop=True)
        brow = tiny.tile([1, G4], BF16, tag="brow")
        nc.vector.scalar_tensor_tensor(brow[:], pb[:], 0.0, bsum2[:],
                                       op0=AO.add, op1=AO.add)
        for k4 in range(4):
            nc.sync.dma_start(bias2k4[k4:k4 + 1, :],
                              brow[0:1, k4 * H:(k4 + 1) * H])

        # ---- phase 2: LSTM2 (bf16 matmuls, fp32 elementwise) ----
        for ch, st in enumerate(chains):
            nc.gpsimd.memset(st["vgsyn2"][:, BP:2 * BP], 0.0)
            st["vgsyn"] = st["vgsyn2"]
            st["mem"] = st["zt"][:]
        cfg2 = dict(wx=wx2s, wh=wh2, thr=float(thr2), is_l2=True,
                    mask4=mask4, bias2k4=bias2k4, spk1_dram=None, edt=F32)
        for t in range(t_run):
            pf = pfpool.tile([128, B], BF16, tag="pf", name="pf")
            nc.sync.dma_start(pf[:], spk1_dram[:, t, :])
            for ch, st in enumerate(chains):
                cfg2["rhs"] = pf[:, ch * BP:(ch + 1) * BP]
                _emit_lstm_step(nc, ch, t, st, cfg2)
        for ch, st in enumerate(chains):
            nc.sync.dma_start(acc2_d[:, ch * BP:(ch + 1) * BP],
                              st["acc2"][:])
    nc.compile()
    return nc


def _prep_host(inputs, t_run):
    """Build per-core input maps from full inputs."""
    x = np.asarray(inputs["x"], np.float32)
    conv_w = np.asarray(inputs["conv_w"], np.float32)
    conv_b = np.asarray(inputs["conv_b"], np.float32)

    def gscale(row512):
        r = row512.copy()
        r[..., 2 * H:3 * H] *= 2.0
        return r

    def tobf(arr):
        return np.ascontiguousarray(arr).astype(ml_dtypes.bfloat16)

    wx1 = np.concatenate(
        [np.asarray(inputs["w_ih1"], np.float32).T,
         (np.asarray(inputs["b_ih1"], np.float32)
          + np.asarray(inputs["b_hh1"], np.float32))[None, :]], axis=0)
    wx1 = tobf(gscale(wx1))
    wh1 = tobf(gscale(np.asarray(inputs["w_hh1"], np.float32).T))
    wx2 = np.ascontiguousarray(gscale(np.asarray(inputs["w_ih2"],
                                                 np.float32).T))
    wh2 = tobf(gscale(np.asarray(inputs["w_hh2"], np.float32).T))
    bsum2 = np.ascontiguousarray(
        gscale((np.asarray(inputs["b_ih2"], np.float32)
                + np.asarray(inputs["b_hh2"], np.float32))[None, :]))
    convw = np.zeros((CP, 96), np.float32)
    for k in range(3):
        convw[:C, k * 32:(k + 1) * 32] = conv_w[:, :, k].T
    convw = tobf(convw)
    thr0 = (1.0 - conv_b)[:, None].astype(np.float32)
    mask4 = tobf(np.kron(np.eye(4, dtype=np.float32),
                         np.ones((1, BP), np.float32)))
    gamma = np.asarray(inputs["bn_gamma"], np.float32)[:, None]
    beta = np.asarray(inputs["bn_beta"], np.float32)[:, None]

    xp = np.zeros((T, L + 2, C), np.float32)
    xp[:, 1:L + 1, :] = x
    in_maps = []
    for k in range(N_CORES):
        xk = xp[:, k * B:k * B + B + 2, :]          # [T, B+2, C]
        xTk = np.zeros((T, CP, B + 2), np.float32)
        xTk[:, :C, :] = xk.transpose(0, 2, 1)
        in_maps.append(dict(
            xT=tobf(xTk), convw=convw, thr0=thr0, wx1=wx1, wh1=wh1,
            wx2=wx2, wh2=wh2, bsum2=bsum2, mask4=mask4,
            gamma=gamma, beta=beta))
    return in_maps


def run(inputs, t_run=T, trace=False):
    thr1 = float(np.asarray(inputs["thr1"]))
    thr2 = float(np.asarray(inputs["thr2"]))
    key = (thr1, thr2, t_run, NCHAINS)
    if key not in _prog_cache:
        _prog_cache[key] = build_program(thr1, thr2, t_run)
    nc = _prog_cache[key]
    in_maps = _prep_host(inputs, t_run)
    res = run_bass_kernel_spmd(nc, in_maps, core_ids=list(range(N_CORES)),
                               trace=trace)
    acc2 = np.concatenate([res.results[k]["acc2"] for k in range(N_CORES)],
                          axis=1)                    # [H, L]
    final_mem = acc2.T / float(t_run)                # [L, H]
    fc_w = np.asarray(inputs["fc_w"], np.float32)
    fc_b = np.asarray(inputs["fc_b"], np.float32)
    out = final_mem @ fc_w.T + fc_b
    return out.astype(np.float32), res


def kernel(**inputs):
    out, _ = run(inputs)
    return out


# revision 10
# speedup vs baseline: 2.6572x; 1.0765x over previous
"""Trainium2 Bass kernel for nn_Net_SLSTM: conv1d -> spiking LSTM -> BN ->
spiking LSTM -> mean -> fc, data-parallel over the batch dim (L=1024) on 8
NeuronCores.

Self-contained: takes FULL inputs, shards internally, returns FULL output.

Key facts used:
- SLSTM with thr=1: mem = sig(o)*tanh(syn) in (-1,1) strictly, so the
  LSTM layers can never spike and reset is always 0. The reset subtract
  is dropped (exact); layer-1 spikes are still computed for the BN
  statistics (they come out zero, but honestly so).
- Layer-1 values only influence the output through those spike counts,
  so all of phase 1 runs in bf16. Phase 2's recurrence does affect the
  output; bf16 matmuls there cost ~1.7e-3 relative error (measured),
  elementwise stays fp32.
- BN over {0,1} spikes: var = mu*(1-mu), so only per-H spike counts are
  needed -> one tiny AllReduce, folded into layer-2's weights/bias.

Structure per core (batch shard B=128, transposed [H, B] layout):
- Input projections, conv and biases are batched over G=4 timesteps into
  per-gate-chunk PSUM banks (big-N matmuls); only the 4 recurrent
  matmuls + sigmoid/tanh/elementwise run per step.
- Gate order [i, f, g, o] with g pre-scaled by 2 on host:
  sigma(2g) -> tanh(g) = 2*sigma(2g)-1 via one cheap tensor_scalar.
"""
import numpy as np
from contextlib import ExitStack

import ml_dtypes
import concourse.bass as bass
import concourse.mybir as mybir
import concourse.tile as tile
from concourse import bacc
from concourse.bass_utils import run_bass_kernel_spmd

F32 = mybir.dt.float32
BF16 = mybir.dt.bfloat16
AO = mybir.AluOpType
AF = mybir.ActivationFunctionType

# Problem shapes (hardcoded per the contract)
T, L, C, H, NCLS = 256, 1024, 14, 128, 7
N_CORES = 8
B = L // N_CORES          # 128 batch rows per core
G4 = 4 * H                # 512

# Tunables
G = 4                     # timesteps batched per PSUM group
XCHUNK = 16               # timesteps of x per input DMA
RING0 = 16                # spk0 ring slots (timesteps)
SRING = 8                 # spike staging ring slots (multiple of G)
BN_EPS = 1e-5

_prog_cache = {}


def _emit_step(nc, t, st, cfg):
    """One LSTM step at time t. PSUM group tile st['ps'] is [128, 4, G, B]
    (gate chunk -> its own bank); mm_x/bias for the whole group were
    already accumulated. Emits the 4 recurrent matmuls + activations +
    elementwise updates."""
    edt = cfg["edt"]
    ps = st["ps"]
    tt = t % G
    u = st["upool"].tile([128, 4 * B], edt, tag="u", name="u")
    # recurrent matmuls, g-chunk first so sigma_g can start early
    order = (2, 0, 1, 3)
    for c in order:
        nc.tensor.matmul(ps[:, c, tt, :], cfg["wh"][:, c * H:(c + 1) * H],
                         st["mem"], start=False, stop=(c == 3))
        if c == 2:
            nc.scalar.activation(u[:, 2 * B:3 * B], ps[:, 2, tt, :],
                                 AF.Sigmoid)
        elif c == 1:
            nc.scalar.activation(u[:, 0:2 * B], ps[:, 0:2, tt, :],
                                 AF.Sigmoid)
        elif c == 3:
            nc.scalar.activation(u[:, 3 * B:4 * B], ps[:, 3, tt, :],
                                 AF.Sigmoid)
    vgsyn = st["vgsyn"]
    # vg = 2*u_g - 1  (= tanh(g))
    nc.vector.tensor_scalar(vgsyn[:, 0:B], u[:, 2 * B:3 * B],
                            2.0, -1.0, op0=AO.mult, op1=AO.add)
    # [t1|t2] = [u_i|u_f] * [vg|syn]
    t12 = st["t12pool"].tile([128, 2 * B], edt, tag="t12", name="t12")
    nc.vector.tensor_tensor(t12[:], u[:, 0:2 * B], vgsyn[:, 0:2 * B],
                            op=AO.mult)
    # syn' = t1 + t2 (into the persistent syn slot)
    nc.vector.tensor_tensor(vgsyn[:, B:2 * B], t12[:, 0:B],
                            t12[:, B:2 * B], op=AO.add)
    w = st["wpool"].tile([128, B], edt, tag="w", name="w")
    nc.scalar.activation(w[:], vgsyn[:, B:2 * B], AF.Tanh)
    # mem' = sig(o)*tanh(syn')   (reset is provably always zero)
    m1 = st["m1pool"].tile([128, B], BF16, tag="m1", name="m1")
    nc.vector.tensor_tensor(m1[:], u[:, 3 * B:4 * B], w[:], op=AO.mult)
    st["mem"] = m1[:]
    if not cfg["is_l2"]:
        # spike = (mem > thr) -> {1,0} bf16 into staging ring;
        # accum_out gives this step's per-H spike count for BN
        slot = t % SRING
        spk_new = st["sring"][:, slot * B:(slot + 1) * B]
        nc.vector.tensor_scalar(spk_new, m1[:], cfg["thr"], 1.0,
                                op0=AO.is_gt, op1=AO.mult,
                                accum_out=st["bnp"][:, t:t + 1])
    else:
        nc.gpsimd.tensor_tensor(st["acc2"][:], st["acc2"][:], m1[:],
                                op=AO.add)


def build_program(thr1, thr2, t_run):
    nc = bacc.Bacc("TRN2", target_bir_lowering=False, debug=False,
                   num_devices=N_CORES)
    # ---- dram I/O ----
    xT_d = nc.dram_tensor("xT", [T, 16, B + 2], BF16, kind="ExternalInput")
    convw_d = nc.dram_tensor("convw", [48, 32], BF16, kind="ExternalInput")
    thr0_d = nc.dram_tensor("thr0", [32, 1], F32, kind="ExternalInput")
    wx1_d = nc.dram_tensor("wx1", [33, G4], BF16, kind="ExternalInput")
    wh1_d = nc.dram_tensor("wh1", [H, G4], BF16, kind="ExternalInput")
    wx2_d = nc.dram_tensor("wx2", [H, G4], F32, kind="ExternalInput")
    wh2_d = nc.dram_tensor("wh2", [H, G4], BF16, kind="ExternalInput")
    bsum2_d = nc.dram_tensor("bsum2", [1, G4], F32, kind="ExternalInput")
    gamma_d = nc.dram_tensor("gamma", [H, 1], F32, kind="ExternalInput")
    beta_d = nc.dram_tensor("beta", [H, 1], F32, kind="ExternalInput")
    acc2_d = nc.dram_tensor("acc2", [H, B], F32, kind="ExternalOutput")
    bnsum_d = nc.dram_tensor("bnsum", [H, 1], F32, kind="ExternalOutput")
    ccw_d = nc.dram_tensor("ccw", [H, 1], F32, kind="ExternalOutput")

    NG = t_run // G
    with ExitStack() as ctx:
        tc = ctx.enter_context(tile.TileContext(nc))
        P = lambda name, bufs, **kw: ctx.enter_context(
            tc.tile_pool(name=name, bufs=bufs, **kw))
        persist = P("persist", 1)
        dram = P("dram", 1, space="DRAM")
        xpool = P("xpool", 3)
        pfpool = P("pfpool", 3)
        gpsum = P("gpsum", 1, space="PSUM")
        psc = P("psc", 2, space="PSUM")
        psb = P("psb", 1, space="PSUM")
        upool = P("upool", 2)
        t12pool = P("t12pool", 2)
        wpool = P("wpool", 2)
        m1pool = P("m1pool", 3)
        tiny = P("tiny", 1)

        # ---- persistent SBUF ----
        convw = persist.tile([48, 32], BF16, tag="convw")
        thr0 = persist.tile([32, 1], F32, tag="thr0")
        wx1 = persist.tile([33, G4], BF16, tag="wx1")
        wh1 = persist.tile([H, G4], BF16, tag="wh1")
        wx2r = persist.tile([H, G4], F32, tag="wx2r")
        wx2s = persist.tile([H, G4], BF16, tag="wx2s")
        wh2 = persist.tile([H, G4], BF16, tag="wh2")
        bsum2 = persist.tile([1, G4], F32, tag="bsum2")
        gamma = persist.tile([H, 1], F32, tag="gamma")
        beta = persist.tile([H, 1], F32, tag="beta")
        brow = persist.tile([1, G4], BF16, tag="brow")
        ones1 = persist.tile([1, G * B], BF16, tag="ones1")
        s0ring = persist.tile([33, RING0 * B], BF16, tag="s0ring")
        spk1_dram = dram.tile([H, T, B], BF16)

        for dst, src in [(convw, convw_d), (thr0, thr0_d), (wx1, wx1_d),
                         (wh1, wh1_d), (wx2r, wx2_d), (wh2, wh2_d),
                         (bsum2, bsum2_d), (gamma, gamma_d),
                         (beta, beta_d)]:
            nc.sync.dma_start(dst[:], src[:])
        nc.gpsimd.memset(s0ring[32:33, :], 1.0)
        nc.gpsimd.memset(ones1[:], 1.0)

        # warm up the collectives path early (result -> ccw output)
        ccin = dram.tile([H, 1], F32)
        ccout = dram.tile([H, 1], F32)
        ccs = tiny.tile([H, 1], F32, tag="ccs")
        nc.gpsimd.memset(ccs[:], 0.0)
        nc.sync.dma_start(ccin[:], ccs[:])
        nc.gpsimd.collective_compute(
            "AllReduce", AO.add, replica_groups=[list(range(N_CORES))],
            ins=[ccin[:]], outs=[ccout[:]])
        nc.sync.dma_start(ccw_d[:], ccout[:])

        # ---- state ----
        st = dict(upool=upool, t12pool=t12pool, wpool=wpool, m1pool=m1pool)
        st["vgsyn1"] = persist.tile([128, 2 * B], BF16, tag="vgsyn1", name="vgsyn1")
        st["vgsyn2"] = persist.tile([128, 2 * B], F32, tag="vgsyn2", name="vgsyn2")
        st["sring"] = persist.tile([128, SRING * B], BF16, tag="sring", name="sring")
        st["bnp"] = persist.tile([128, t_run], F32, tag="bnp", name="bnp")
        st["acc2"] = persist.tile([128, B], F32, tag="acc2", name="acc2")
        zt = persist.tile([128, B], BF16, tag="zt")
        nc.gpsimd.memset(zt[:], 0.0)
        nc.gpsimd.memset(st["vgsyn1"][:, B:2 * B], 0.0)
        nc.gpsimd.memset(st["acc2"][:], 0.0)
        st["mem"] = zt[:]
        st["vgsyn"] = st["vgsyn1"]

        # ---- phase 1: conv + LSTM1 (all bf16) ----
        cfg1 = dict(wh=wh1, thr=float(thr1), is_l2=False, edt=BF16)
        x48 = None
        for t in range(t_run):
            if t % XCHUNK == 0:
                x48 = xpool.tile([48, XCHUNK, B], BF16, tag="x48",
                                 name="x48")
                for k in range(3):
                    nc.sync.dma_start(
                        x48[16 * k:16 * (k + 1), :, :],
                        xT_d[t:t + XCHUNK, :, k:k + B].rearrange(
                            "t c l -> c t l"))
            if t % G == 0:
                # conv for the G steps of this group -> heaviside -> ring
                pcv = psc.tile([32, G * B], F32, tag="pc", name="pcv")
                tt0 = t % XCHUNK
                nc.tensor.matmul(pcv[:], convw[:],
                                 x48[:, tt0:tt0 + G, :], start=True,
                                 stop=True)
                slot0 = t % RING0
                nc.vector.tensor_scalar(
                    s0ring[0:32, slot0 * B:(slot0 + G) * B], pcv[:],
                    thr0[:], None, op0=AO.is_gt)
                # group PSUM: bias-free; x-side projections for G steps
                ps = gpsum.tile([128, 4, G, B], F32, tag="ps", name="ps")
                st["ps"] = ps
                for c in range(4):
                    nc.tensor.matmul(
                        ps[:, c, :, :], wx1[:, c * H:(c + 1) * H],
                        s0ring[0:33, slot0 * B:(slot0 + G) * B],
                        start=True, stop=False)
            _emit_step(nc, t, st, cfg1)
            if (t + 1) % G == 0:
                s0 = (t + 1 - G) % SRING
                src = st["sring"][:, s0 * B:(s0 + G) * B]
                nc.sync.dma_start(
                    spk1_dram[:, t + 1 - G:t + 1, :],
                    src.rearrange("p (s b) -> p s b", b=B))

        # ---- BN stats + allreduce + weight fold (fp32, tiny) ----
        r = tiny.tile([H, 1], F32, tag="r0")
        nc.vector.tensor_reduce(r[:], st["bnp"][:], mybir.AxisListType.X,
                                AO.add)
        bnin = dram.tile([H, 1], F32)
        bnout = dram.tile([H, 1], F32)
        nc.sync.dma_start(bnin[:], r[:])
        nc.gpsimd.collective_compute(
            "AllReduce", AO.add, replica_groups=[list(range(N_CORES))],
            ins=[bnin[:]], outs=[bnout[:]])
        stot = tiny.tile([H, 1], F32, tag="stot")
        nc.sync.dma_start(stot[:], bnout[:])
        nc.sync.dma_start(bnsum_d[:], bnout[:])
        mu = tiny.tile([H, 1], F32, tag="mu")
        nc.vector.tensor_scalar_mul(mu[:], stot[:], 1.0 / (t_run * L))
        om = tiny.tile([H, 1], F32, tag="om")
        nc.vector.tensor_scalar(om[:], mu[:], -1.0, 1.0,
                                op0=AO.mult, op1=AO.add)
        var = tiny.tile([H, 1], F32, tag="var")
        nc.vector.tensor_tensor(var[:], mu[:], om[:], op=AO.mult)
        xve = tiny.tile([H, 1], F32, tag="xve")
        nc.vector.tensor_scalar_add(xve[:], var[:], BN_EPS)
        epsb = tiny.tile([H, 1], F32, tag="epsb")
        nc.gpsimd.memset(epsb[:], BN_EPS)
        y1 = tiny.tile([H, 1], F32, tag="y1")
        nc.scalar.activation(y1[:], var[:], AF.Sqrt, bias=epsb[:])
        # one Newton step: y2 = 0.5*(y1 + x/y1); a = gamma/y2
        ry = tiny.tile([H, 1], F32, tag="ry")
        nc.vector.reciprocal(ry[:], y1[:])
        z = tiny.tile([H, 1], F32, tag="z")
        nc.vector.tensor_tensor(z[:], xve[:], ry[:], op=AO.mult)
        y2 = tiny.tile([H, 1], F32, tag="y2")
        nc.vector.tensor_tensor(y2[:], y1[:], z[:], op=AO.add)
        nc.vector.tensor_scalar_mul(y2[:], y2[:], 0.5)
        rinv = tiny.tile([H, 1], F32, tag="rinv")
        nc.vector.reciprocal(rinv[:], y2[:])
        a = tiny.tile([H, 1], F32, tag="a")
        nc.vector.tensor_tensor(a[:], gamma[:], rinv[:], op=AO.mult)
        cm = tiny.tile([H, 1], F32, tag="cm")
        nc.vector.tensor_tensor(cm[:], mu[:], a[:], op=AO.mult)
        cvec = tiny.tile([H, 1], F32, tag="cvec")
        nc.vector.tensor_tensor(cvec[:], beta[:], cm[:], op=AO.subtract)
        # wx2s = wx2r * a (per-partition, bf16 out); brow = c^T wx2r + bsum2
        nc.vector.tensor_scalar_mul(wx2s[:], wx2r[:], a[:])
        pb = psb.tile([1, G4], F32, tag="pb")
        nc.tensor.matmul(pb[:], cvec[:], wx2r[:], start=True, stop=True)
        nc.vector.scalar_tensor_tensor(brow[:], pb[:], 0.0, bsum2[:],
                                       op0=AO.add, op1=AO.add)

        # ---- phase 2: LSTM2 (bf16 matmuls, fp32 elementwise) ----
        nc.gpsimd.memset(st["vgsyn2"][:, B:2 * B], 0.0)
        st["vgsyn"] = st["vgsyn2"]
        st["mem"] = zt[:]
        cfg2 = dict(wh=wh2, thr=float(thr2), is_l2=True, edt=F32)
        for t in range(t_run):
            if t % G == 0:
                pf = pfpool.tile([128, G, B], BF16, tag="pf", name="pf")
                nc.sync.dma_start(pf[:], spk1_dram[:, t:t + G, :])
                ps = gpsum.tile([128, 4, G, B], F32, tag="ps", name="ps")
                st["ps"] = ps
                for c in range(4):
                    nc.tensor.matmul(ps[:, c, :, :],
                                     brow[0:1, c * H:(c + 1) * H],
                                     ones1[0:1, :], start=True, stop=False)
                    nc.tensor.matmul(ps[:, c, :, :],
                                     wx2s[:, c * H:(c + 1) * H],
                                     pf[:].rearrange("p s b -> p (s b)"),
                                     start=False, stop=False)
            _emit_step(nc, t, st, cfg2)
        nc.sync.dma_start(acc2_d[:], st["acc2"][:])
    nc.compile()
    return nc


def _prep_host(inputs, t_run):
    """Build per-core input maps from full inputs."""
    x = np.asarray(inputs["x"], np.float32)
    conv_w = np.asarray(inputs["conv_w"], np.float32)
    conv_b = np.asarray(inputs["conv_b"], np.float32)

    def gscale(row512):
        r = row512.copy()
        r[..., 2 * H:3 * H] *= 2.0
        return r

    def tobf(arr):
        return np.ascontiguousarray(arr).astype(ml_dtypes.bfloat16)

    wx1 = np.concatenate(
        [np.asarray(inputs["w_ih1"], np.float32).T,
         (np.asarray(inputs["b_ih1"], np.float32)
          + np.asarray(inputs["b_hh1"], np.float32))[None, :]], axis=0)
    wx1 = tobf(gscale(wx1))
    wh1 = tobf(gscale(np.asarray(inputs["w_hh1"], np.float32).T))
    wx2 = np.ascontiguousarray(gscale(np.asarray(inputs["w_ih2"],
                                                 np.float32).T))
    wh2 = tobf(gscale(np.asarray(inputs["w_hh2"], np.float32).T))
    bsum2 = np.ascontiguousarray(
        gscale((np.asarray(inputs["b_ih2"], np.float32)
                + np.asarray(inputs["b_hh2"], np.float32))[None, :]))
    convw = np.zeros((48, 32), np.float32)
    for k in range(3):
        convw[16 * k:16 * k + C, :] = conv_w[:, :, k].T
    convw = tobf(convw)
    thr0 = (1.0 - conv_b)[:, None].astype(np.float32)
    gamma = np.asarray(inputs["bn_gamma"], np.float32)[:, None]
    beta = np.asarray(inputs["bn_beta"], np.float32)[:, None]

    xp = np.zeros((T, L + 2, C), np.float32)
    xp[:, 1:L + 1, :] = x
    in_maps = []
    for k in range(N_CORES):
        xk = xp[:, k * B:k * B + B + 2, :]          # [T, B+2, C]
        xTk = np.zeros((T, 16, B + 2), np.float32)
        xTk[:, :C, :] = xk.transpose(0, 2, 1)
        in_maps.append(dict(
            xT=tobf(xTk), convw=convw, thr0=thr0, wx1=wx1, wh1=wh1,
            wx2=wx2, wh2=wh2, bsum2=bsum2, gamma=gamma, beta=beta))
    return in_maps


def run(inputs, t_run=T, trace=False):
    thr1 = float(np.asarray(inputs["thr1"]))
    thr2 = float(np.asarray(inputs["thr2"]))
    key = (thr1, thr2, t_run)
    if key not in _prog_cache:
        _prog_cache[key] = build_program(thr1, thr2, t_run)
    nc = _prog_cache[key]
    in_maps = _prep_host(inputs, t_run)
    res = run_bass_kernel_spmd(nc, in_maps, core_ids=list(range(N_CORES)),
                               trace=trace)
    acc2 = np.concatenate([res.results[k]["acc2"] for k in range(N_CORES)],
                          axis=1)                    # [H, L]
    final_mem = acc2.T / float(t_run)                # [L, H]
    fc_w = np.asarray(inputs["fc_w"], np.float32)
    fc_b = np.asarray(inputs["fc_b"], np.float32)
    out = final_mem @ fc_w.T + fc_b
    return out.astype(np.float32), res


def kernel(**inputs):
    out, _ = run(inputs)
    return out
